# revision 29
# baseline (speedup 1.0000x reference)
"""Trainium2 Bass kernel for the differentiable circle renderer.

Math: the sequential over-composite
    canvas <- canvas*(1-g_i) + col_i*g_i,   g_i = alpha_i * sigmoid((r_i-d_i)/0.01)
unrolls (Abel summation) to
    canvas_c = K_c + sum_i D_ic * S_i,      S_i = prod_{j>=i} (1-g_j)
with D_ic = col_{i-1,c}-col_ic and K_c = col_{N-1,c}.  Suffix products go
through log space (S = exp(Tri @ ln(1-alpha*m))).

Accelerations vs the direct render (rel-err budget 2e-2, this lands ~1e-3):

1. Low-res render + on-device bilinear upsample.  SOFTNESS=0.01 makes every
   mask edge a ~100-px sigmoid ramp, so the canvas is smooth at the 8-px
   scale.  Render a 129x129 global grid (lo pixel j at position 8j/1023, so
   hi pixel x=8j+k interpolates lo j..j+1 with weight k/8 exactly) and
   upsample 8x per axis.  All transcendental work drops 64x.
2. No sqrt pass: m = sigmoid(a_i*(r_i^2 - d^2)), a_i = min(50/r_i, 2000)
   (slope-matched at the circle edge; validated numerically).
3. Per-tile circle culling + partition packing.  Each core renders a
   16-lo-row slab; split it into two column-half tiles.  Per tile, drop
   circles that don't touch it (their color diffs telescope into the next
   kept circle's D) and drop every circle older than the point where the
   suffix occlusion sum of fully-covering later circles exceeds 10 (their
   suffix products are < e^-10 everywhere on the tile; the anchor color of
   the newest occluded circle seeds the telescoped D).  Max kept is 50 <=
   64, so both tiles pack into the 128 partitions and the free (pixel) dim
   halves: engine time scales with free size only.
4. The whole sigmoid argument z is precomputed on host in fp16 per packed
   (tile-circle, tile-pixel) slot; three DMA chunks ride three rings.
5. Vertical (row) upsample = PE matmul over partitions (17 lo rows -> 128 hi
   rows), with +K_c folded in as an all-ones extra contraction row whose
   moving-operand row holds the per-tile K_c.  The horizontal step Delta/8
   comes from a second matmul with a 1/8-scaled stationary.
6. Horizontal upsample in phase-major layout G[c, k, j] (canvas at x=8j+k):
   chains G[c,k,:] = G[c,k-1,:] + Delta8[c,:] are contiguous packed-fp16
   DVE ops, the fp16 output DMA halves the bandwidth-bound tail, and the
   host unshard step casts to f32 and permutes columns back to x=8j+k.

ACT phases are ordered sigmoid -> {ln, exp, copy} so only the two
activation-table loads occur (ln+exp+copy share natural_log_exp_and_others).
"""

import sys

sys.path.insert(0, "/opt/trn_rl_repo")

import numpy as np

CANVAS = 1024
N = 128
NCORES = 8
ROWS = CANVAS // NCORES  # 128 hi-res rows per core
W = CANVAS
F = 8  # upsample factor per axis
LC = CANVAS // F + 1  # 129 lo cols
LO = ROWS // F + 1  # 17 lo rows per core
TC = 65  # lo cols per column-half tile
K = 64  # packed circles per tile
LPP = LO * TC  # 1105 packed lo pixels (per tile)
A_MAX = 2000.0  # cap on sigmoid sharpness a_i = 50/r_i
THETA = 10.0  # occlusion-culling threshold on the suffix log-sum
MARGIN = 0.07  # touch-culling distance margin (sigmoid(-7) ~ 9e-4)

_CACHE = {}


def split_multiwaits(nc, max_waits=1):
    """Walrus in this container rejects >max_waits sem waits on one
    instruction; hoist extras onto standalone NoOps placed just before."""
    from concourse import mybir

    ctr = 0
    for bb in nc.main_func.blocks:
        new = []
        for inst in bb.instructions:
            si = inst.sync_info
            if si is not None and len(si.on_wait) > max_waits:
                waits = list(si.on_wait)
                extra, keep = waits[:-max_waits], waits[-max_waits:]
                for wt in extra:
                    ctr += 1
                    nop = mybir.InstNoOp(
                        name=f"waitsplit_{ctr}",
                        opcode="NoOp",
                        engine=inst.engine,
                        sync_info=mybir.SyncInfo(on_wait=[wt], on_update=[]),
                    )
                    new.append(nop)
                inst.sync_info = mybir.SyncInfo(
                    on_wait=keep, on_update=list(si.on_update)
                )
            new.append(inst)
        bb.instructions = new
    return ctr


def insert_table_loads(nc):
    """Pre-place InstLoadActFuncSet so walrus adopts our table choice:
    serve Ln AND Exp from natural_log_exp_and_others instead of a greedy
    split that reloads 1.28us tables on every transition."""
    import bass_rust as _bass_rust
    from concourse.hw_specs import get_activation_tables
    from concourse import mybir

    tables = get_activation_tables(nc.m.arch)
    strip = {mybir.ActivationFunctionType.Exp, mybir.ActivationFunctionType.Ln}
    curated = [
        (name, set(s) if name == "natural_log_exp_and_others" else set(s) - strip)
        for name, s in tables.items()
    ]
    _bass_rust.insert_act_table_loads(nc, curated)


def build_nc():
    """Build the SPMD Bass program (identical on all cores; data differs)."""
    import concourse.bass as bass
    import concourse.tile as tile
    from concourse import mybir

    f32 = mybir.dt.float32
    f16 = mybir.dt.float16
    AF = mybir.ActivationFunctionType
    ALU = mybir.AluOpType

    nc = bass.Bass()
    Z_d = nc.declare_dram_parameter("Z", [2 * K, LPP], f16, isOutput=False)
    NA_d = nc.declare_dram_parameter("NA", [2 * K, 1], f32, isOutput=False)
    # block-diag TRI [128,128] and packed DST [128,6] side by side
    TD_d = nc.declare_dram_parameter("TD", [2 * K, 2 * K + 6], f16, isOutput=False)
    # VST [18,128] and VST/8 packed side by side
    VV_d = nc.declare_dram_parameter("VV", [LO + 1, 2 * ROWS], f16, isOutput=False)
    KR_d = nc.declare_dram_parameter("KR", [1, 3 * LC], f16, isOutput=False)
    # fp16 output in phase-major column order [c, r, k*128+j]; the host
    # unshard step casts to f32 and permutes columns back to x=8j+k.
    OUT_d = nc.declare_dram_parameter("OUT", [3, ROWS, W], f16, isOutput=True)

    zc = [0, 256, 680, LPP]  # exp/recip/ln chunk bounds (DMA-aligned)
    gc = [0, 512, 1024, LPP]  # Tri/exp/D pipeline chunk bounds

    with tile.TileContext(nc) as tc:
        with (
            tc.tile_pool(name="const", bufs=1) as cpool,
            tc.tile_pool(name="sl", bufs=2, space="PSUM") as slp,
            tc.tile_pool(name="cl", bufs=2, space="PSUM") as clp,
            tc.tile_pool(name="yv", bufs=1, space="PSUM") as yvp,
            tc.tile_pool(name="dv", bufs=1, space="PSUM") as dvp,
        ):
            Zt = cpool.tile([2 * K, LPP], f16)
            NA = cpool.tile([2 * K, 1], f32)
            TD = cpool.tile([2 * K, 2 * K + 6], f16)
            VV = cpool.tile([LO + 1, 2 * ROWS], f16)
            X = cpool.tile([LO + 1, 3 * LC], f16)
            m = cpool.tile([2 * K, LPP], f32)
            L = cpool.tile([2 * K, LPP], f16)
            S = cpool.tile([2 * K, LPP], f16)
            CLS = cpool.tile([6, LPP], f16)
            XD = cpool.tile([LO + 1, 3 * (LC - 1)], f16)
            D8 = cpool.tile([ROWS, 3 * (LC - 1)], f16)
            G = cpool.tile([ROWS, 3 * W], f16)

            with tc.tile_wait_until(0):
                # z chunks gate the sigmoid phase AND each DMA ring moves
                # only ~60-100GB/s: put the three chunks on three different
                # issuing engines so the transfers ride separate rings.
                for (c0, c1), eng in zip(
                    zip(zc[:-1], zc[1:]), (nc.sync, nc.gpsimd, nc.scalar)
                ):
                    eng.dma_start(Zt[:, c0:c1], Z_d[:, c0:c1])
                nc.gpsimd.dma_start(NA[:], NA_d[:])
                nc.gpsimd.dma_start(TD[:], TD_d[:])
                nc.sync.dma_start(VV[:], VV_d[:])
                nc.sync.dma_start(X[LO : LO + 1, :], KR_d[:])
            # Phase 1: m = sigmoid(z)  [table: sigmoid_and_others]
            with tc.tile_wait_until(1):
                for c0, c1 in zip(zc[:-1], zc[1:]):
                    nc.scalar.activation(m[:, c0:c1], Zt[:, c0:c1], AF.Sigmoid)
            # Phase 2: L = ln(1 - alpha*m) -> fp16  [table: ln+exp set]
            with tc.tile_wait_until(2):
                for c0, c1 in zip(zc[:-1], zc[1:]):
                    nc.scalar.activation(
                        L[:, c0:c1], m[:, c0:c1], AF.Ln, scale=NA[:, 0:1], bias=1.0
                    )
            # Phase 3: per chunk: SL = Tri@L (block-diag suffix sums per
            # tile); S = exp(SL); Clo = D@S ([6,w]: rgb of tile A then B);
            # bounce Clo PSUM->SBUF (fp16) on the DVE.
            with tc.tile_wait_until(3):
                for c0, c1 in zip(gc[:-1], gc[1:]):
                    w = c1 - c0
                    sl = slp.tile([2 * K, w], f32)
                    nc.tensor.matmul(
                        sl[:], TD[:, 0 : 2 * K], L[:, c0:c1], start=True, stop=True
                    )
                    nc.scalar.activation(S[:, c0:c1], sl[:], AF.Exp)
                    cl = clp.tile([6, w], f32)
                    nc.tensor.matmul(
                        cl[:],
                        TD[:, 2 * K : 2 * K + 6],
                        S[:, c0:c1],
                        start=True,
                        stop=True,
                    )
                    nc.vector.tensor_copy(CLS[:, c0:c1], cl[:])
            # Phase 4: rearrange CLS [ch(+3 for tile B), row x 65] into
            # X [row, ch x 129] (tile A supplies lo cols 0..64, tile B cols
            # 65..128), six DMAs over three rings; lo-col deltas; vertical-
            # interp matmuls; Delta8 PSUM->SBUF copy on ACT (Copy shares the
            # ln/exp table set).
            with tc.tile_wait_until(4):
                CLS3 = CLS[:].rearrange("p (j x) -> p j x", j=LO)
                # six drains over the two cheap-issue engines (a scalar-
                # engine dma_start costs ~1.4us of ACT time)
                engs = (nc.gpsimd, nc.sync)
                for ch in range(3):
                    engs[ch % 2].dma_start(
                        X[0:LO, ch * LC : ch * LC + TC], CLS[ch : ch + 1, :]
                    )
                    engs[(ch + 1) % 2].dma_start(
                        X[0:LO, ch * LC + TC : (ch + 1) * LC],
                        CLS3[ch + 3 : ch + 4, :, 1:TC],
                    )
                X3 = X[:].rearrange("p (c x) -> p c x", c=3)
                XD3 = XD[:].rearrange("p (c x) -> p c x", c=3)
                nc.vector.tensor_tensor(
                    XD3[:, :, :], X3[:, :, 1:LC], X3[:, :, 0 : LC - 1], op=ALU.subtract
                )
                yv = yvp.tile([ROWS, 3 * LC], f32)
                nc.tensor.matmul(yv[:], VV[:, 0:ROWS], X[:], start=True, stop=True)
                dv = dvp.tile([ROWS, 3 * (LC - 1)], f32)
                nc.tensor.matmul(
                    dv[:], VV[:, ROWS : 2 * ROWS], XD[:], start=True, stop=True
                )
                nc.scalar.activation(D8[:], dv[:], AF.Copy, bias=0.0, scale=1.0)
            # Phase 5: horizontal chains, phase-major: G[c, k, j] holds the
            # canvas at x=8j+k, so every op reads/writes contiguous fp16
            # blocks (DVE packed fast path).  G[c,k,:] = G[c,k-1,:] + D8[c,:].
            with tc.tile_wait_until(5):
                Gp = G[:].rearrange("p (c k j) -> p c k j", c=3, k=F)
                Y3 = yv[:].rearrange("p (c x) -> p c x", c=3)
                D3 = D8[:].rearrange("p (c x) -> p c x", c=3)
                nc.vector.tensor_copy(Gp[:, :, 0, :], Y3[:, :, 0 : LC - 1])
                for k in range(1, F):
                    nc.vector.tensor_tensor(
                        Gp[:, :, k, :], Gp[:, :, k - 1, :], D3[:, :, :],
                        op=ALU.add,
                    )
                engs = (nc.sync, nc.gpsimd, nc.scalar)
                for ch in range(3):
                    engs[ch].dma_start(
                        OUT_d[ch, :, :], G[:, ch * W : (ch + 1) * W]
                    )
    insert_table_loads(nc)
    split_multiwaits(nc)
    return nc


def host_inputs(centers, radii, colors):
    """Per-core input maps with per-(slab, column-half) circle culling."""
    centers = np.asarray(centers, np.float64)
    radii = np.asarray(radii, np.float64)
    colors = np.asarray(colors, np.float64)
    cx, cy = centers[:, 0], centers[:, 1]
    r = radii
    alpha = colors[:, 3]
    rgb = colors[:, :3]
    a = np.minimum(50.0 / r, A_MAX)
    pos = np.arange(LC, dtype=np.float64) * F / (CANVAS - 1)

    # vertical interp weights: hi row rl <- lo rows rl//8, rl//8+1
    VV = np.zeros((LO + 1, 2 * ROWS), np.float16)
    rl = np.arange(ROWS)
    j0 = rl // F
    wv = (rl - j0 * F) / F
    VST = np.zeros((LO + 1, ROWS), np.float64)
    VST[j0, rl] = 1.0 - wv
    VST[j0 + 1, rl] += wv
    VST[LO, :] = 1.0  # all-ones row: adds K_c (X row 17 holds per-tile K)
    VV[:, :ROWS] = VST.astype(np.float16)
    VV[:, ROWS:] = (VST / F).astype(np.float16)

    TRI = np.zeros((2 * K, 2 * K), np.float16)
    TRI[:K, :K] = np.tril(np.ones((K, K)))  # TRI[j,i]=1 iff j>=i, per tile
    TRI[K:, K:] = np.tril(np.ones((K, K)))

    def cull(y0, y1, x0, x1):
        dx = np.maximum(np.maximum(x0 - cx, cx - x1), 0.0)
        dy = np.maximum(np.maximum(y0 - cy, cy - y1), 0.0)
        dmin = np.hypot(dx, dy)
        dxm = np.maximum(np.abs(cx - x0), np.abs(x1 - cx))
        dym = np.maximum(np.abs(cy - y0), np.abs(y1 - cy))
        dmax = np.hypot(dxm, dym)
        touch = dmin <= r + MARGIN
        full = dmax <= r - 0.05
        m_min = 1.0 / (1.0 + np.exp(-100.0 * (r - dmax)))
        occ = np.where(full, -np.log1p(-alpha * m_min), 0.0)
        suf = np.cumsum(occ[::-1])[::-1]
        suf = np.concatenate([suf[1:], [0.0]])  # over j>i
        keep = touch & (suf < THETA)
        kl = np.where(keep)[0]
        occl = np.where(suf >= THETA)[0]
        anchor = int(occl.max()) if len(occl) else -1
        assert 1 <= len(kl) <= K, f"culled count {len(kl)} out of range"
        return kl, anchor

    in_maps = []
    for kcore in range(NCORES):
        ys_k = np.arange(16 * kcore, 16 * kcore + LO, dtype=np.float64) * F / (
            CANVAS - 1
        )
        y0, y1 = kcore / 8.0, (kcore + 1) / 8.0 + F / (CANVAS - 1)
        Z = np.full((2 * K, LPP), -30.0, np.float64)  # pad rows: sigmoid ~ 0
        NAp = np.zeros((2 * K, 1), np.float32)
        DST = np.zeros((2 * K, 6), np.float16)
        KR = np.zeros((1, 3 * LC), np.float16)
        for half in range(2):
            x0, x1 = half / 2.0, (half + 1) / 2.0 + F / (CANVAS - 1)
            kl, anchor = cull(y0, y1, x0, x1)
            nk = len(kl)
            p0 = half * K + (K - nk)  # front-pad
            xs_t = pos[64 * half : 64 * half + TC]
            d2 = (xs_t[None, None, :] - cx[kl, None, None]) ** 2 + (
                ys_k[None, :, None] - cy[kl, None, None]
            ) ** 2
            z = a[kl, None, None] * (r[kl, None, None] ** 2 - d2)
            Z[p0 : p0 + nk] = np.maximum(z, -30.0).reshape(nk, LPP)
            NAp[p0 : p0 + nk, 0] = -alpha[kl]
            prev = np.concatenate([[anchor], kl[:-1]])
            D = np.where(prev[:, None] >= 0, rgb[prev], 1.0) - rgb[kl]
            DST[p0 : p0 + nk, 3 * half : 3 * half + 3] = D.astype(np.float16)
            Kc = rgb[kl[-1]].astype(np.float16)
            for c in range(3):
                if half == 0:
                    KR[0, c * LC : c * LC + TC] = Kc[c]
                else:
                    KR[0, c * LC + TC : (c + 1) * LC] = Kc[c]
        TD = np.concatenate([TRI, DST], axis=1)
        in_maps.append(
            {
                "Z": Z.astype(np.float16),
                "NA": NAp,
                "TD": np.ascontiguousarray(TD),
                "VV": VV,
                "KR": KR,
            }
        )
    return in_maps


def kernel(centers, radii, colors, trace=False):
    from concourse.bass_utils import run_bass_kernel_spmd

    if "nc" not in _CACHE:
        _CACHE["nc"] = build_nc()
    nc = _CACHE["nc"]
    in_maps = host_inputs(centers, radii, colors)
    res = run_bass_kernel_spmd(nc, in_maps, list(range(NCORES)), trace=trace)
    _CACHE["last_result"] = res
    # device columns are phase-major [k*128+j]; permute back to x=8j+k
    parts = []
    for k in range(NCORES):
        raw = np.asarray(res.results[k]["OUT"], np.float32)
        parts.append(
            raw.reshape(3, ROWS, F, W // F).transpose(0, 1, 3, 2).reshape(3, ROWS, W)
        )
    out = np.concatenate(parts, axis=1)
    return np.ascontiguousarray(out, dtype=np.float32)


# revision 35
# speedup vs baseline: 1.0927x; 1.0927x over previous
"""Trainium2 Bass kernel for the differentiable circle renderer.

Math: the sequential over-composite
    canvas <- canvas*(1-g_i) + col_i*g_i,   g_i = alpha_i * sigmoid((r_i-d_i)/0.01)
unrolls (Abel summation) to
    canvas_c = K_c + sum_i D_ic * S_i,      S_i = prod_{j>=i} (1-g_j)
with D_ic = col_{i-1,c}-col_ic and K_c = col_{N-1,c}.  Suffix products go
through log space (S = exp(Tri @ ln(1-alpha*m))).

Accelerations vs the direct render (rel-err budget 2e-2, this lands ~1e-3):

1. Low-res render + on-device bilinear upsample.  SOFTNESS=0.01 makes every
   mask edge a ~100-px sigmoid ramp, so the canvas is smooth at the 8-px
   scale.  Render a 129x129 global grid (lo pixel j at position 8j/1023, so
   hi pixel x=8j+k interpolates lo j..j+1 with weight k/8 exactly) and
   upsample 8x per axis.  All transcendental work drops 64x.
2. No sqrt pass: m = sigmoid(a_i*(r_i^2 - d^2)), a_i = min(50/r_i, 2000)
   (slope-matched at the circle edge; validated numerically).
3. Per-tile circle culling + partition packing.  Each core renders a
   16-lo-row slab; split it into two column-half tiles.  Per tile, drop
   circles that don't touch it (their color diffs telescope into the next
   kept circle's D) and drop every circle older than the point where the
   suffix occlusion sum of fully-covering later circles exceeds 10 (their
   suffix products are < e^-10 everywhere on the tile; the anchor color of
   the newest occluded circle seeds the telescoped D).  Max kept is 50 <=
   64, so both tiles pack into the 128 partitions and the free (pixel) dim
   halves: engine time scales with free size only.
4. The whole sigmoid argument z is precomputed on host in fp16 per packed
   (tile-circle, tile-pixel) slot; three DMA chunks ride three rings.
5. Vertical (row) upsample = PE matmul over partitions (17 lo rows -> 128 hi
   rows), with +K_c folded in as an all-ones extra contraction row whose
   moving-operand row holds the per-tile K_c.  The horizontal step Delta/8
   comes from a second matmul with a 1/8-scaled stationary.
6. Horizontal upsample in phase-major layout G[c, k, j] (canvas at x=8j+k):
   chains G[c,k,:] = G[c,k-1,:] + Delta8[c,:] are contiguous packed-fp16
   DVE ops, the fp16 output DMA halves the bandwidth-bound tail, and the
   host unshard step casts to f32 and permutes columns back to x=8j+k.

ACT phases are ordered sigmoid -> {ln, exp, copy} so only the two
activation-table loads occur (ln+exp+copy share natural_log_exp_and_others).
"""

import sys

sys.path.insert(0, "/opt/trn_rl_repo")

import numpy as np

CANVAS = 1024
N = 128
NCORES = 8
ROWS = CANVAS // NCORES  # 128 hi-res rows per core
W = CANVAS
F = 16  # upsample factor per axis
LC = CANVAS // F + 1  # 65 lo cols
LO = ROWS // F + 1  # 9 lo rows per core
RPC = ROWS // F  # 8 lo rows between cores
TC = LC // 2 + 1  # 33 lo cols per column-half tile
K = 64  # packed circles per tile
LPP = LO * TC  # 297 packed lo pixels (per tile)
A_MAX = 2000.0  # cap on sigmoid sharpness a_i = 50/r_i
THETA = 10.0  # occlusion-culling threshold on the suffix log-sum
MARGIN = 0.07  # touch-culling distance margin (sigmoid(-7) ~ 9e-4)

_CACHE = {}


def split_multiwaits(nc, max_waits=1):
    """Walrus in this container rejects >max_waits sem waits on one
    instruction; hoist extras onto standalone NoOps placed just before."""
    from concourse import mybir

    ctr = 0
    for bb in nc.main_func.blocks:
        new = []
        for inst in bb.instructions:
            si = inst.sync_info
            if si is not None and len(si.on_wait) > max_waits:
                waits = list(si.on_wait)
                extra, keep = waits[:-max_waits], waits[-max_waits:]
                for wt in extra:
                    ctr += 1
                    nop = mybir.InstNoOp(
                        name=f"waitsplit_{ctr}",
                        opcode="NoOp",
                        engine=inst.engine,
                        sync_info=mybir.SyncInfo(on_wait=[wt], on_update=[]),
                    )
                    new.append(nop)
                inst.sync_info = mybir.SyncInfo(
                    on_wait=keep, on_update=list(si.on_update)
                )
            new.append(inst)
        bb.instructions = new
    return ctr


def insert_table_loads(nc):
    """Pre-place InstLoadActFuncSet so walrus adopts our table choice:
    serve Ln AND Exp from natural_log_exp_and_others instead of a greedy
    split that reloads 1.28us tables on every transition."""
    import bass_rust as _bass_rust
    from concourse.hw_specs import get_activation_tables
    from concourse import mybir

    tables = get_activation_tables(nc.m.arch)
    strip = {mybir.ActivationFunctionType.Exp, mybir.ActivationFunctionType.Ln}
    curated = [
        (name, set(s) if name == "natural_log_exp_and_others" else set(s) - strip)
        for name, s in tables.items()
    ]
    _bass_rust.insert_act_table_loads(nc, curated)


def build_nc():
    """Build the SPMD Bass program (identical on all cores; data differs)."""
    import concourse.bass as bass
    import concourse.tile as tile
    from concourse import mybir

    f32 = mybir.dt.float32
    f16 = mybir.dt.float16
    AF = mybir.ActivationFunctionType
    ALU = mybir.AluOpType

    nc = bass.Bass()
    Z_d = nc.declare_dram_parameter("Z", [2 * K, LPP], f16, isOutput=False)
    NA_d = nc.declare_dram_parameter("NA", [2 * K, 1], f32, isOutput=False)
    # block-diag TRI [128,128] and packed DST [128,6] side by side
    TD_d = nc.declare_dram_parameter("TD", [2 * K, 2 * K + 6], f16, isOutput=False)
    # VST [18,128] and VST/8 packed side by side
    VV_d = nc.declare_dram_parameter("VV", [LO + 1, 2 * ROWS], f16, isOutput=False)
    KR_d = nc.declare_dram_parameter("KR", [1, 3 * LC], f16, isOutput=False)
    # fp16 output in phase-major column order [c, r, k*128+j]; the host
    # unshard step casts to f32 and permutes columns back to x=8j+k.
    OUT_d = nc.declare_dram_parameter("OUT", [3, ROWS, W], f16, isOutput=True)

    zc = [0, 128, LPP]  # sigmoid/ln chunk bounds (DMA-aligned)
    gc = [0, LPP]  # Tri/exp/D pipeline chunk bounds (297 f32 = one bank)

    with tile.TileContext(nc) as tc:
        with (
            tc.tile_pool(name="const", bufs=1) as cpool,
            tc.tile_pool(name="sl", bufs=2, space="PSUM") as slp,
            tc.tile_pool(name="cl", bufs=2, space="PSUM") as clp,
            tc.tile_pool(name="yv", bufs=1, space="PSUM") as yvp,
            tc.tile_pool(name="dv", bufs=1, space="PSUM") as dvp,
        ):
            Zt = cpool.tile([2 * K, LPP], f16)
            NA = cpool.tile([2 * K, 1], f32)
            TD = cpool.tile([2 * K, 2 * K + 6], f16)
            VV = cpool.tile([LO + 1, 2 * ROWS], f16)
            X = cpool.tile([LO + 1, 3 * LC], f16)
            m = cpool.tile([2 * K, LPP], f32)
            L = cpool.tile([2 * K, LPP], f16)
            S = cpool.tile([2 * K, LPP], f16)
            CLS = cpool.tile([6, LPP], f16)
            XD = cpool.tile([LO + 1, 3 * (LC - 1)], f16)
            D8 = cpool.tile([ROWS, 3 * (LC - 1)], f16)
            G = cpool.tile([ROWS, 3 * W], f16)

            with tc.tile_wait_until(0):
                # z chunks gate the sigmoid phase AND each DMA ring moves
                # only ~60-100GB/s: put the three chunks on three different
                # issuing engines so the transfers ride separate rings.
                for (c0, c1), eng in zip(
                    zip(zc[:-1], zc[1:]), (nc.sync, nc.gpsimd, nc.scalar)
                ):
                    eng.dma_start(Zt[:, c0:c1], Z_d[:, c0:c1])
                nc.gpsimd.dma_start(NA[:], NA_d[:])
                nc.gpsimd.dma_start(TD[:], TD_d[:])
                nc.sync.dma_start(VV[:], VV_d[:])
                nc.sync.dma_start(X[LO : LO + 1, :], KR_d[:])
            # Phase 1: m = sigmoid(z)  [table: sigmoid_and_others]
            with tc.tile_wait_until(1):
                for c0, c1 in zip(zc[:-1], zc[1:]):
                    nc.scalar.activation(m[:, c0:c1], Zt[:, c0:c1], AF.Sigmoid)
            # Phase 2: L = ln(1 - alpha*m) -> fp16  [table: ln+exp set]
            with tc.tile_wait_until(2):
                for c0, c1 in zip(zc[:-1], zc[1:]):
                    nc.scalar.activation(
                        L[:, c0:c1], m[:, c0:c1], AF.Ln, scale=NA[:, 0:1], bias=1.0
                    )
            # Phase 3: per chunk: SL = Tri@L (block-diag suffix sums per
            # tile); S = exp(SL); Clo = D@S ([6,w]: rgb of tile A then B);
            # bounce Clo PSUM->SBUF (fp16) on the DVE.
            with tc.tile_wait_until(3):
                for c0, c1 in zip(gc[:-1], gc[1:]):
                    w = c1 - c0
                    sl = slp.tile([2 * K, w], f32)
                    nc.tensor.matmul(
                        sl[:], TD[:, 0 : 2 * K], L[:, c0:c1], start=True, stop=True
                    )
                    nc.scalar.activation(S[:, c0:c1], sl[:], AF.Exp)
                    cl = clp.tile([6, w], f32)
                    nc.tensor.matmul(
                        cl[:],
                        TD[:, 2 * K : 2 * K + 6],
                        S[:, c0:c1],
                        start=True,
                        stop=True,
                    )
                    nc.vector.tensor_copy(CLS[:, c0:c1], cl[:])
            # Phase 4: rearrange CLS [ch(+3 for tile B), row x 65] into
            # X [row, ch x 129] (tile A supplies lo cols 0..64, tile B cols
            # 65..128), six DMAs over three rings; lo-col deltas; vertical-
            # interp matmuls; Delta8 PSUM->SBUF copy on ACT (Copy shares the
            # ln/exp table set).
            with tc.tile_wait_until(4):
                CLS3 = CLS[:].rearrange("p (j x) -> p j x", j=LO)
                # six drains over the two cheap-issue engines (a scalar-
                # engine dma_start costs ~1.4us of ACT time)
                engs = (nc.gpsimd, nc.sync)
                for ch in range(3):
                    engs[ch % 2].dma_start(
                        X[0:LO, ch * LC : ch * LC + TC], CLS[ch : ch + 1, :]
                    )
                    engs[(ch + 1) % 2].dma_start(
                        X[0:LO, ch * LC + TC : (ch + 1) * LC],
                        CLS3[ch + 3 : ch + 4, :, 1:TC],
                    )
                X3 = X[:].rearrange("p (c x) -> p c x", c=3)
                XD3 = XD[:].rearrange("p (c x) -> p c x", c=3)
                nc.vector.tensor_tensor(
                    XD3[:, :, :], X3[:, :, 1:LC], X3[:, :, 0 : LC - 1], op=ALU.subtract
                )
                yv = yvp.tile([ROWS, 3 * LC], f32)
                nc.tensor.matmul(yv[:], VV[:, 0:ROWS], X[:], start=True, stop=True)
                dv = dvp.tile([ROWS, 3 * (LC - 1)], f32)
                nc.tensor.matmul(
                    dv[:], VV[:, ROWS : 2 * ROWS], XD[:], start=True, stop=True
                )
                nc.scalar.activation(D8[:], dv[:], AF.Copy, bias=0.0, scale=1.0)
            # Phase 5: horizontal chains, phase-major: G[c, k, j] holds the
            # canvas at x=8j+k, so every op reads/writes contiguous fp16
            # blocks (DVE packed fast path).  G[c,k,:] = G[c,k-1,:] + D8[c,:].
            with tc.tile_wait_until(5):
                Gp = G[:].rearrange("p (c k j) -> p c k j", c=3, k=F)
                Y3 = yv[:].rearrange("p (c x) -> p c x", c=3)
                D3 = D8[:].rearrange("p (c x) -> p c x", c=3)
                nc.vector.tensor_copy(Gp[:, :, 0, :], Y3[:, :, 0 : LC - 1])
                for k in range(1, F):
                    nc.vector.tensor_tensor(
                        Gp[:, :, k, :], Gp[:, :, k - 1, :], D3[:, :, :],
                        op=ALU.add,
                    )
                engs = (nc.sync, nc.gpsimd, nc.scalar)
                for ch in range(3):
                    engs[ch].dma_start(
                        OUT_d[ch, :, :], G[:, ch * W : (ch + 1) * W]
                    )
    insert_table_loads(nc)
    split_multiwaits(nc)
    return nc


def host_inputs(centers, radii, colors):
    """Per-core input maps with per-(slab, column-half) circle culling."""
    centers = np.asarray(centers, np.float64)
    radii = np.asarray(radii, np.float64)
    colors = np.asarray(colors, np.float64)
    cx, cy = centers[:, 0], centers[:, 1]
    r = radii
    alpha = colors[:, 3]
    rgb = colors[:, :3]
    a = np.minimum(50.0 / r, A_MAX)
    pos = np.arange(LC, dtype=np.float64) * F / (CANVAS - 1)

    # vertical interp weights: hi row rl <- lo rows rl//8, rl//8+1
    VV = np.zeros((LO + 1, 2 * ROWS), np.float16)
    rl = np.arange(ROWS)
    j0 = rl // F
    wv = (rl - j0 * F) / F
    VST = np.zeros((LO + 1, ROWS), np.float64)
    VST[j0, rl] = 1.0 - wv
    VST[j0 + 1, rl] += wv
    VST[LO, :] = 1.0  # all-ones row: adds K_c (X row 17 holds per-tile K)
    VV[:, :ROWS] = VST.astype(np.float16)
    VV[:, ROWS:] = (VST / F).astype(np.float16)

    TRI = np.zeros((2 * K, 2 * K), np.float16)
    TRI[:K, :K] = np.tril(np.ones((K, K)))  # TRI[j,i]=1 iff j>=i, per tile
    TRI[K:, K:] = np.tril(np.ones((K, K)))

    def cull(y0, y1, x0, x1):
        dx = np.maximum(np.maximum(x0 - cx, cx - x1), 0.0)
        dy = np.maximum(np.maximum(y0 - cy, cy - y1), 0.0)
        dmin = np.hypot(dx, dy)
        dxm = np.maximum(np.abs(cx - x0), np.abs(x1 - cx))
        dym = np.maximum(np.abs(cy - y0), np.abs(y1 - cy))
        dmax = np.hypot(dxm, dym)
        touch = dmin <= r + MARGIN
        full = dmax <= r - 0.05
        m_min = 1.0 / (1.0 + np.exp(-100.0 * (r - dmax)))
        occ = np.where(full, -np.log1p(-alpha * m_min), 0.0)
        suf = np.cumsum(occ[::-1])[::-1]
        suf = np.concatenate([suf[1:], [0.0]])  # over j>i
        keep = touch & (suf < THETA)
        kl = np.where(keep)[0]
        occl = np.where(suf >= THETA)[0]
        anchor = int(occl.max()) if len(occl) else -1
        assert 1 <= len(kl) <= K, f"culled count {len(kl)} out of range"
        return kl, anchor

    in_maps = []
    for kcore in range(NCORES):
        ys_k = np.arange(RPC * kcore, RPC * kcore + LO, dtype=np.float64) * F / (
            CANVAS - 1
        )
        y0, y1 = kcore / 8.0, (kcore + 1) / 8.0 + F / (CANVAS - 1)
        Z = np.full((2 * K, LPP), -30.0, np.float64)  # pad rows: sigmoid ~ 0
        NAp = np.zeros((2 * K, 1), np.float32)
        DST = np.zeros((2 * K, 6), np.float16)
        KR = np.zeros((1, 3 * LC), np.float16)
        for half in range(2):
            x0, x1 = half / 2.0, (half + 1) / 2.0 + F / (CANVAS - 1)
            kl, anchor = cull(y0, y1, x0, x1)
            nk = len(kl)
            p0 = half * K + (K - nk)  # front-pad
            xs_t = pos[(TC - 1) * half : (TC - 1) * half + TC]
            d2 = (xs_t[None, None, :] - cx[kl, None, None]) ** 2 + (
                ys_k[None, :, None] - cy[kl, None, None]
            ) ** 2
            z = a[kl, None, None] * (r[kl, None, None] ** 2 - d2)
            Z[p0 : p0 + nk] = np.maximum(z, -30.0).reshape(nk, LPP)
            NAp[p0 : p0 + nk, 0] = -alpha[kl]
            prev = np.concatenate([[anchor], kl[:-1]])
            D = np.where(prev[:, None] >= 0, rgb[prev], 1.0) - rgb[kl]
            DST[p0 : p0 + nk, 3 * half : 3 * half + 3] = D.astype(np.float16)
            Kc = rgb[kl[-1]].astype(np.float16)
            for c in range(3):
                if half == 0:
                    KR[0, c * LC : c * LC + TC] = Kc[c]
                else:
                    KR[0, c * LC + TC : (c + 1) * LC] = Kc[c]
        TD = np.concatenate([TRI, DST], axis=1)
        in_maps.append(
            {
                "Z": Z.astype(np.float16),
                "NA": NAp,
                "TD": np.ascontiguousarray(TD),
                "VV": VV,
                "KR": KR,
            }
        )
    return in_maps


def kernel(centers, radii, colors, trace=False):
    from concourse.bass_utils import run_bass_kernel_spmd

    if "nc" not in _CACHE:
        _CACHE["nc"] = build_nc()
    nc = _CACHE["nc"]
    in_maps = host_inputs(centers, radii, colors)
    res = run_bass_kernel_spmd(nc, in_maps, list(range(NCORES)), trace=trace)
    _CACHE["last_result"] = res
    # device columns are phase-major [k*128+j]; permute back to x=8j+k
    parts = []
    for k in range(NCORES):
        raw = np.asarray(res.results[k]["OUT"], np.float32)
        parts.append(
            raw.reshape(3, ROWS, F, W // F).transpose(0, 1, 3, 2).reshape(3, ROWS, W)
        )
    out = np.concatenate(parts, axis=1)
    return np.ascontiguousarray(out, dtype=np.float32)
